# revision 6
# baseline (speedup 1.0000x reference)
"""Trainium2 Bass kernel for nn_BasicFlowLayer (deformable-conv flow layer).

Contract: kernel(**inputs) takes FULL unsharded numpy inputs (as produced by
setup_inputs) and returns the FULL [4, 64, 128, 128] float32 output.

Sharding: 8 cores = 4 samples x 2 row-halves (64 output rows each).
All convs recompute halo rows; the deformable gather reads real neighbor
rows, so the sharded result equals the unsharded one.

Deformable sampling uses the exact triangle-window identity
    bilinear(x, s) = sum_{p in Z} relu(1-|s-p|) * x[p]
which for |offset| < 1 needs only the static 3x3 window around each tap.

v2 redesign (347.8us vs the 390.3us 72-partition window pipeline):
  - 128-partition packing of the 576 (tap,group,chan) deform planes:
    tile A = taps 0-7 x 8 groups x 2 chan-halves (the other 4 channels
    ride the free dim), tile B = tap 8 x (group,chan) on 64 partitions.
    Window products use all 128 DVE lanes (1.8x fewer cycles) and the
    deform matmuls drop from 16 to 10 N=512 chunks per window (PE
    deform 246us -> 154us; matmul cost is N x cycles regardless of
    contraction depth, so fewer accumulation chunks is the only lever).
  - u planes (tri-window weights, shared across channels) are
    replicated across partitions by DMA with partition-STRIDED
    DESTINATION APs ([0::2]/[1::2] for tile A, [j::8] for tile B),
    batched per band ([*, 9, 1024]). Stride-0 source APs do NOT
    replicate on real HW (verified: the dim is collapsed); strided
    destinations do.
  - per-band host-prepared x replicas (halo rows duplicated) make each
    band's xa ONE contiguous-descriptor DMA; xin is host-bf16.
  - scheduling around the in-order sequencers (the hard part): every
    queue stalls at its head, so any instruction whose deps resolve
    late blocks everything behind it on that engine. The cross-band
    chain om(i+2)-matmuls -> ACT evac+tris (band-i tail, ~1 band of
    slack) then folds -> u-products -> replication DMAs (band-(i+1)
    head/early fillers) keeps each link's deps ~a band old when its
    queue reaches it. Window products are emitted one window AHEAD of
    their matmuls; each band's PSUM evac is deferred into the NEXT
    band's stream (an evac at band end would park the ACT queue on the
    last-matmul sem and delay the next om/tri chain behind it); DMAs
    with unavoidable WAR parks (ra_hi) are issued LAST on SP so
    nothing queues behind them. Conv1/conv2 blocks are PE fillers
    placed late in each band (their DVE/ACT evacs would otherwise park
    those queues mid-chain). Keeping the PE backlog continuous also
    keeps its p-state at 2.4GHz (the cost model drops to 1.2GHz for
    3us after any idle).
  Real-ISA constraints found along the way: TensorScalarPtr is invalid
  on the Pool engine; a matmul writes at most one PSUM bank (N<=512
  fp32); ACT PSUM reads need 32-aligned start partitions; fp8
  DoubleRow mis-computes on real HW and DoubleRowSwInterleave fails
  neuronx-cc codegen (plain fp8 is correct but has no cost advantage);
  ACT Lrelu has slope 0.01, not the 0.1 this model needs; Pool
  tensor_tensor runs at 0.42 efficiency (~2.3ns/free-elem vs DVE 2x
  bf16 at ~0.52), so Pool only carries the small tap-8 products.
"""

import numpy as np

import concourse.bacc as bacc
import concourse.tile as tile
import concourse.mybir as mybir
from concourse import bass_utils

FP32 = mybir.dt.float32
BF16 = mybir.dt.bfloat16

NF = 64
DG = 8
CG = NF // DG
B, H, W = 4, 128, 128
K = 3
TAPS = K * K
NCORES = 8
NR = H // 2          # output rows per core
DBLK = 8             # deform row-block
NBAND = NR // DBLK
CBLK = 4             # conv row-block (4*128 = 512 = max fp32 matmul N)
GK = DG * TAPS       # 72
OMC = 3 * GK         # 216 om output channels
PXB = DBLK * W       # 1024 pixels per band

DDT = BF16           # deform-stage data dtype


def _tap(i):
    return i // K - 1, i % K - 1  # ky, kx


def build_program():
    nc = bacc.Bacc("TRN2", target_bir_lowering=False, debug=False,
                   enable_asserts=True, num_devices=NCORES)

    xin_d = nc.dram_tensor("xin", [2 * NF, NR + 6, W + 2], BF16, kind="ExternalInput")
    # per-band tap-shifted x replicas, 128-partition packing:
    #   A: partition p=(k*16 + g*2 + ch), free (cb in 4, jj in 10, w in 132)
    #      holds image[g*8 + ch*4 + cb] shifted by tap k (zero outside)
    #   B: partition q=(g*8 + c), tap 8
    nbxa_d = nc.dram_tensor("nbxa", [NBAND, 128, 4, DBLK + 2, W + 4], BF16,
                            kind="ExternalInput")
    nbxb_d = nc.dram_tensor("nbxb", [NBAND, NF, DBLK + 2, W + 4], BF16,
                            kind="ExternalInput")
    w1_d = nc.dram_tensor("w1t", [2 * NF, TAPS, NF], FP32, kind="ExternalInput")
    w2p_d = nc.dram_tensor("w2p", [2 * NF, K, NF], FP32, kind="ExternalInput")
    w2s_d = nc.dram_tensor("w2s", [NF, K, NF], FP32, kind="ExternalInput")
    womp_d = nc.dram_tensor("womp", [2 * NF, K, OMC], FP32, kind="ExternalInput")
    woms_d = nc.dram_tensor("woms", [NF, K, OMC], FP32, kind="ExternalInput")
    wda_d = nc.dram_tensor("wda", [128, 4, NF], BF16, kind="ExternalInput")
    wdb_d = nc.dram_tensor("wdb", [NF, NF], BF16, kind="ExternalInput")
    rm1_d = nc.dram_tensor("rmask1", [2 * NF, NR + 6, 1], BF16, kind="ExternalInput")
    rm2_d = nc.dram_tensor("rmask2", [2 * NF, NR + 4, 1], BF16, kind="ExternalInput")
    b1_d = nc.dram_tensor("b1", [NF, 1], FP32, kind="ExternalInput")
    b2_d = nc.dram_tensor("b2", [NF, 1], FP32, kind="ExternalInput")
    bom_d = nc.dram_tensor("bom", [OMC, 1], FP32, kind="ExternalInput")
    bd_d = nc.dram_tensor("bd", [NF, 1], FP32, kind="ExternalInput")
    out_d = nc.dram_tensor("out", [NF, NR, W], FP32, kind="ExternalOutput")

    with tile.TileContext(nc) as tc:
        build_kernel(tc, xin_d, nbxa_d, nbxb_d, w1_d, w2p_d, w2s_d, womp_d,
                     woms_d, wda_d, wdb_d, b1_d, b2_d, bom_d, bd_d, out_d,
                     rm1_d, rm2_d)
    nc.compile()
    return nc


def _evac_lrelu_pair(nc, ev, opair, rows, psum_ap, bias_ap, nr):
    """lrelu(psum+bias) -> opair[0:64] at col 1.. plus the col-shifted copy
    at [64:128] col 0.. (K-stacking for tap pairs)."""
    AF = mybir.ActivationFunctionType
    t = ev.tile([NF, CBLK, W], BF16, tag="lrelu_t")
    nc.scalar.activation(t[:, :nr, :], psum_ap, AF.Identity,
                         bias=bias_ap, scale=1.0)
    nc.vector.scalar_tensor_tensor(
        out=opair[0:NF, rows, 1:1 + W],
        in0=t[:, :nr, :], scalar=0.1, in1=t[:, :nr, :],
        op0=mybir.AluOpType.mult, op1=mybir.AluOpType.max)
    nc.scalar.copy(opair[NF:2 * NF, rows, 0:W], opair[0:NF, rows, 1:1 + W])


def build_kernel(tc, xin_d, nbxa_d, nbxb_d, w1_d, w2p_d, w2s_d, womp_d,
                 woms_d, wda_d, wdb_d, b1_d, b2_d, bom_d, bd_d, out_d,
                 rm1_d, rm2_d):
    nc = tc.nc
    AF = mybir.ActivationFunctionType
    ALU = mybir.AluOpType

    with tc.tile_pool(name="persist", bufs=1) as pp, \
         tc.tile_pool(name="ev", bufs=1) as ev:

        wda = pp.tile([128, 4, NF], DDT)
        nc.sync.dma_start(wda[:], wda_d[:])
        wdb = pp.tile([NF, NF], DDT)
        nc.sync.dma_start(wdb[:], wdb_d[:])
        bd_s = pp.tile([NF, 1], FP32)
        nc.sync.dma_start(bd_s[:], bd_d[:])
        aw = pp.tile([NF, 1], FP32, tag="actwarm")
        for fn_ in (AF.Identity, AF.Abs, AF.Relu, AF.Sigmoid):
            nc.scalar.activation(aw[:], bd_s[:], fn_, bias=0.0, scale=1.0)

        with tc.tile_pool(name="p_o1", bufs=1) as p1:
            o1 = p1.tile([2 * NF, NR + 6, W + 2], DDT)
            # only pad columns 0 and W+1 of the lower half are ever read;
            # every other cell is written before any read.
            nc.vector.memset(o1[0:NF, :, 0:1], 0.0)
            nc.vector.memset(o1[0:NF, :, W + 1:W + 2], 0.0)
            rm1 = p1.tile([2 * NF, NR + 6, 1], DDT)
            nc.sync.dma_start(rm1[:], rm1_d[:])

            from contextlib import ExitStack
            with ExitStack() as _st:
                p0 = _st.enter_context(tc.tile_pool(name="p_xin", bufs=1))
                psA = _st.enter_context(tc.tile_pool(name="psA", bufs=1, space="PSUM"))
                p2 = _st.enter_context(tc.tile_pool(name="p_o2", bufs=1))
                pw2 = _st.enter_context(tc.tile_pool(name="p_w2", bufs=1))
                psB = _st.enter_context(tc.tile_pool(name="psB", bufs=1, space="PSUM"))
                pwom = _st.enter_context(tc.tile_pool(name="p_wom", bufs=1))
                psC = _st.enter_context(tc.tile_pool(name="psC", bufs=1, space="PSUM"))
                pfld = _st.enter_context(tc.tile_pool(name="p_fld", bufs=1))
                prep = _st.enter_context(tc.tile_pool(name="p_rep", bufs=2))
                ppl = _st.enter_context(tc.tile_pool(name="p_pl", bufs=1))
                pu = _st.enter_context(tc.tile_pool(name="p_u", bufs=1))
                purl = _st.enter_context(tc.tile_pool(name="p_url", bufs=2))
                purm = _st.enter_context(tc.tile_pool(name="p_urm", bufs=2))
                purh = _st.enter_context(tc.tile_pool(name="p_urh", bufs=1))
                purb = _st.enter_context(tc.tile_pool(name="p_urb", bufs=2))
                pw = _st.enter_context(tc.tile_pool(name="p_w", bufs=2))
                pwb = _st.enter_context(tc.tile_pool(name="p_wb", bufs=2))
                pos = _st.enter_context(tc.tile_pool(name="p_os", bufs=1))
                psD = _st.enter_context(tc.tile_pool(name="psD", bufs=2, space="PSUM"))

                w1 = p0.tile([2 * NF, TAPS, NF], DDT)
                nc.gpsimd.dma_start(w1[:], w1_d[:])
                xin = p0.tile([2 * NF, NR + 6, W + 2], DDT)
                # first chunk small so conv1 block 0 starts early
                for r0_, r1_ in ((0, 8), (8, 26), (26, 48), (48, NR + 6)):
                    nc.scalar.dma_start(xin[:, r0_:r1_, :],
                                        xin_d[:, r0_:r1_, :])
                b1 = p0.tile([NF, 1], FP32)
                nc.sync.dma_start(b1[:], b1_d[:])
                o2 = p2.tile([2 * NF, NR + 4, W + 2], DDT)
                nc.vector.memset(o2[0:NF, :, 0:1], 0.0)
                nc.vector.memset(o2[0:NF, :, W + 1:W + 2], 0.0)
                rm2 = p2.tile([2 * NF, NR + 4, 1], DDT)
                nc.sync.dma_start(rm2[:], rm2_d[:])
                w2p = pw2.tile([2 * NF, K, NF], DDT)
                nc.gpsimd.dma_start(w2p[:], w2p_d[:])
                w2s = pw2.tile([NF, K, NF], DDT)
                nc.gpsimd.dma_start(w2s[:], w2s_d[:])
                b2 = pw2.tile([NF, 1], FP32)
                nc.sync.dma_start(b2[:], b2_d[:])
                womp = pwom.tile([2 * NF, K, OMC], DDT)
                nc.gpsimd.dma_start(womp[:], womp_d[:])
                woms = pwom.tile([NF, K, OMC], DDT)
                nc.gpsimd.dma_start(woms[:], woms_d[:])
                bomf = []
                for f in range(3):
                    bf_ = pwom.tile([GK, 1], FP32, tag=f"bomf{f}")
                    nc.sync.dma_start(bf_[:], bom_d[f * GK:(f + 1) * GK])
                    bomf.append(bf_)

                nrows1 = NR + 4
                nblk1 = (nrows1 + CBLK - 1) // CBLK
                emitted1 = 0

                def emit_conv1_through(last):
                    nonlocal emitted1
                    while emitted1 <= min(last, nblk1 - 1):
                        bi = emitted1
                        t0 = bi * CBLK
                        nr = min(CBLK, nrows1 - t0)
                        acc = psA.tile([NF, CBLK, W], FP32, tag="accA",
                                       name=f"accA_{bi}")
                        for it, (ky, kx) in enumerate(map(_tap, range(TAPS))):
                            rhs = xin[:, t0 + 1 + ky: t0 + 1 + ky + nr,
                                      1 + kx: 1 + kx + W]
                            nc.tensor.matmul(acc[:, :nr, :], w1[:, it, :], rhs,
                                             start=(it == 0), stop=(it == TAPS - 1))
                        rows = slice(t0 + 1, t0 + 1 + nr)
                        _evac_lrelu_pair(nc, ev, o1, rows, acc[:, :nr, :],
                                         b1[:, 0:1], nr)
                        if bi in (0, nblk1 - 1):
                            nc.vector.tensor_mul(
                                o1[0:NF, rows, :], o1[0:NF, rows, :],
                                rm1[0:NF, rows, :].broadcast_to([NF, nr, W + 2]))
                            nc.vector.tensor_mul(
                                o1[NF:, rows, 0:W], o1[NF:, rows, 0:W],
                                rm1[NF:, rows, :].broadcast_to([NF, nr, W]))
                        emitted1 += 1

                nrows2 = NR + 2
                nblk2 = (nrows2 + CBLK - 1) // CBLK
                emitted = 0

                def emit_conv2_through(last):
                    nonlocal emitted
                    while emitted <= min(last, nblk2 - 1):
                        bj = emitted
                        t0 = bj * CBLK
                        nr = min(CBLK, nrows2 - t0)
                        acc = psB.tile([NF, CBLK, W], FP32, tag="accB",
                                       name=f"accB_{bj}")
                        for a, ky in enumerate((-1, 0, 1)):
                            rows = slice(t0 + 2 + ky, t0 + 2 + ky + nr)
                            nc.tensor.matmul(acc[:, :nr, :], w2p[:, a, :],
                                             o1[:, rows, 0:W],
                                             start=(a == 0), stop=False)
                            nc.tensor.matmul(acc[:, :nr, :], w2s[:, a, :],
                                             o1[0:NF, rows, 2:2 + W],
                                             start=False, stop=(a == 2))
                        rows = slice(t0 + 1, t0 + 1 + nr)
                        _evac_lrelu_pair(nc, ev, o2, rows, acc[:, :nr, :],
                                         b2[:, 0:1], nr)
                        if bj in (0, nblk2 - 1):
                            nc.vector.tensor_mul(
                                o2[0:NF, rows, :], o2[0:NF, rows, :],
                                rm2[0:NF, rows, :].broadcast_to([NF, nr, W + 2]))
                            nc.vector.tensor_mul(
                                o2[NF:, rows, 0:W], o2[NF:, rows, 0:W],
                                rm2[NF:, rows, :].broadcast_to([NF, nr, W]))
                        emitted += 1

                def om_matmuls(acc, s0, f):
                    """one field of the om conv over a band, emitted as
                    N=512 half-band matmuls (one PSUM bank per write)"""
                    mlo, mhi = f * GK, (f + 1) * GK
                    for h in range(2):
                        t0 = s0 + h * CBLK
                        hacc = acc[:, h * CBLK:(h + 1) * CBLK, :]
                        for a, ky in enumerate((-1, 0, 1)):
                            rows = slice(t0 + 2 + ky, t0 + 2 + ky + CBLK)
                            nc.tensor.matmul(hacc, womp[:, a, mlo:mhi],
                                             o2[:, rows, 0:W],
                                             start=(a == 0), stop=False)
                            nc.tensor.matmul(hacc, woms[:, a, mlo:mhi],
                                             o2[0:NF, rows, 2:2 + W],
                                             start=False, stop=(a == 2))

                flds = {}
                accs = {}

                xas = {}

                def emit_xa(i):
                    """band i's x-replica DMAs (contiguous full-band
                    descriptors). Issued at band (i-1)'s HEAD: there the
                    WAR dep (band i-2's window reads) is already resolved,
                    so the SP queue never parks on it."""
                    s0 = i * DBLK
                    xa = prep.tile([128, 4, DBLK + 2, W + 4], DDT,
                                   tag="xrepa", name=f"xa_{s0}")
                    nc.sync.dma_start(xa[:], nbxa_d[i])
                    xb = prep.tile([NF, DBLK + 2, W + 4], DDT,
                                   tag="xrepb", name=f"xb_{s0}")
                    nc.sync.dma_start(xb[:], nbxb_d[i])
                    xas[i] = (xa, xb)

                def emit_om_pre(i):
                    s0 = i * DBLK
                    fb = []
                    for f in range(3):
                        fld = pfld.tile([GK, DBLK, W], DDT, tag=f"fld{f}",
                                        name=f"fld{f}_{s0}")
                        fb.append(fld)
                    flds[i] = fb

                def emit_om_field(i, f):
                    """one om field for band i. [72]-partition psum chunk:
                    the ACT engine needs PSUM reads starting at a 32-aligned
                    partition, so each field evacuates from partition 0."""
                    s0 = i * DBLK
                    fb = flds[i]
                    accc = psC.tile([GK, DBLK, W], FP32, tag="accC",
                                    name=f"accC{f}_{s0}")
                    om_matmuls(accc, s0, f)
                    func = AF.Sigmoid if f == 2 else AF.Identity
                    nc.scalar.activation(fb[f][:], accc[:], func,
                                         bias=bomf[f][:, 0:1], scale=1.0)

                def emit_om(i):
                    emit_om_pre(i)
                    for f in range(3):
                        emit_om_field(i, f)

                tris = {}

                def emit_tris(i, axis):
                    """triangle weights for one axis of band i (ACT only).
                    tri(v,-1)=relu(-v), tri(v,0)=1-|v|, tri(v,+1)=relu(v).
                    Interleaved with the om field evacs so the ACT chain for
                    band i finishes well before band i's windows start."""
                    s0 = i * DBLK
                    fb = flds[i]
                    src_ap = fb[0] if axis == "y" else fb[1]
                    dst = tris.setdefault(i, {"y": [], "x": []})[axis]
                    a = ev.tile([GK, DBLK, W], DDT, tag="abs",
                                name=f"abs{axis}_{s0}")
                    nc.scalar.activation(a[:], src_ap[:], AF.Abs,
                                         bias=0.0, scale=1.0)
                    w0 = ppl.tile([GK, DBLK, W], DDT, tag=f"w{axis}0",
                                  name=f"w{axis}0_{s0}")
                    nc.scalar.activation(w0[:], a[:], AF.Identity,
                                         bias=1.0, scale=-1.0)
                    wp = ppl.tile([GK, DBLK, W], DDT, tag=f"w{axis}p",
                                  name=f"w{axis}p_{s0}")
                    nc.scalar.activation(wp[:], src_ap[:], AF.Relu,
                                         bias=0.0, scale=1.0)
                    # wm written in-place into the (now dead) field tile
                    wm = src_ap
                    nc.scalar.activation(wm[:], src_ap[:], AF.Relu,
                                         bias=0.0, scale=-1.0)
                    dst.extend((wm, w0, wp))

                def emit_folds(i):
                    """mask fold into the wy planes. Stays on DVE: the
                    fold sits on the serial cross-band chain (tri -> fold ->
                    u -> DMA), and Pool's 3.5x slower tensor_tensor lengthens
                    that chain more than DVE's queue pressure costs."""
                    fb = flds[i]
                    wy = tris[i]["y"]
                    for e in range(3):
                        nc.vector.tensor_mul(wy[e][:], fb[2][:], wy[e][:])

                urep = {}

                def emit_u(i, part):
                    """u products (3 windows per call) + their replication
                    DMAs:  A: p=(kg*2+ch) <- u[kg] (2 strided DMAs/chunk)
                           B: q=(g*8+c)  <- u8[g]  (8 strided DMAs)
                    Split into 3 staggered slots so the first chunk's DMAs
                    complete well before band i starts."""
                    s0 = i * DBLK
                    wy, wx = tris[i]["y"], tris[i]["x"]
                    if part == 0:
                        ua = pu.tile([GK, TAPS, DBLK, W], DDT, tag="uall",
                                     name=f"uall_{s0}")
                        urep[i] = [ua]
                    ua = urep[i][0]
                    for nj in range(3 * part, 3 * part + 3):
                        ey, ex = nj // 3, nj % 3
                        nc.vector.tensor_mul(ua[:, nj, :, :],
                                             wy[ey][:], wx[ex][:])
                    if part == 0:
                        ra_lo = purl.tile([128, 3, DBLK, W], DDT, tag="urepal",
                                          name=f"urepal_{s0}")
                        nc.sync.dma_start(ra_lo[0::2], ua[0:64, 0:3])
                        nc.sync.dma_start(ra_lo[1::2], ua[0:64, 0:3])
                        urep[i].append(ra_lo)
                    elif part == 1:
                        ra_md = purm.tile([128, 3, DBLK, W], DDT, tag="urepam",
                                          name=f"urepam_{s0}")
                        nc.sync.dma_start(ra_md[0::2], ua[0:64, 3:6])
                        nc.sync.dma_start(ra_md[1::2], ua[0:64, 3:6])
                        urep[i].append(ra_md)
                    else:
                        rb = purb.tile([NF, TAPS, DBLK, W], DDT, tag="urepb",
                                       name=f"urepb_{s0}")
                        for j in range(8):
                            nc.sync.dma_start(rb[j::8], ua[64:GK])
                        # ra_hi is bufs=1: its WAR wait (prev band's window-8
                        # product read) parks on the SP queue ~til band end,
                        # so it is issued LAST — nothing queues behind it
                        # until the next band's emit_u.
                        ra_hi = purh.tile([128, 3, DBLK, W], DDT, tag="urepah",
                                          name=f"urepah_{s0}")
                        nc.sync.dma_start(ra_hi[0::2], ua[0:64, 6:TAPS])
                        nc.sync.dma_start(ra_hi[1::2], ua[0:64, 6:TAPS])
                        urep[i].extend((rb, ra_hi))

                def emit_back(i, fillers):
                    """window products + all deform matmuls + evac.
                    Filler closures (conv block / om chunk / next front)
                    are emitted between windows so the PE keeps backlog."""
                    s0 = i * DBLK
                    xa, xb = xas[i]
                    _, ra_lo, ra_md, rb, ra_hi = urep[i]
                    acc = psD.tile([NF, DBLK, W], FP32, tag="accD",
                                   name=f"accD_{s0}")
                    def prods(nj):
                        """window products: Pool (tap8) + DVE (taps 0-7,
                        per half). Emitted one window AHEAD of their
                        matmuls: the PE queue is in-order, so a stalled
                        window matmul would block the fillers behind it."""
                        ey, ex = nj // 3, nj % 3
                        ra, ji = ((ra_lo, nj) if nj < 3 else
                                  (ra_md, nj - 3) if nj < 6 else
                                  (ra_hi, nj - 6))
                        pb = pwb.tile([NF, DBLK, W], DDT, tag="pb",
                                      name=f"pb_{s0}_{nj}")
                        nc.gpsimd.tensor_mul(
                            pb[:], rb[:, nj, :, :],
                            xb[:, ey:ey + DBLK, 1 + ex:1 + ex + W])
                        pas = []
                        for h in range(2):
                            hr = slice(h * CBLK, (h + 1) * CBLK)
                            pa = pw.tile([128, 4, CBLK, W], DDT, tag=f"pa{h}",
                                         name=f"pa{h}_{s0}_{nj}")
                            ub = ra[:, ji, None, hr, :].broadcast_to(
                                [128, 4, CBLK, W])
                            nc.vector.tensor_mul(
                                pa[:], ub,
                                xa[:, :, ey + h * CBLK:ey + h * CBLK + CBLK,
                                   1 + ex:1 + ex + W])
                            pas.append(pa)
                        return pas, pb

                    cur = prods(0)
                    for nj in range(TAPS):
                        nxt = prods(nj + 1) if nj + 1 < TAPS else None
                        pas, pb = cur
                        for h in range(2):
                            hacc = acc[:, h * CBLK:(h + 1) * CBLK, :]
                            hr = slice(h * CBLK, (h + 1) * CBLK)
                            for cb in range(4):
                                nc.tensor.matmul(hacc, wda[:, cb, :],
                                                 pas[h][:, cb, :, :],
                                                 start=(nj == 0 and cb == 0),
                                                 stop=False)
                            nc.tensor.matmul(hacc, wdb[:], pb[:, hr, :],
                                             start=False,
                                             stop=(nj == TAPS - 1))
                            if nj == TAPS - 1 and i == NBAND - 1:
                                # last band: evacuate each half as soon as
                                # its accumulation stops, so the h0 evac +
                                # out DMA overlap h1's final matmuls and
                                # the program drain starts sooner
                                osb = pos.tile([NF, CBLK, W], FP32,
                                               tag="osb",
                                               name=f"osbL{h}_{s0}")
                                nc.scalar.activation(
                                    osb[:], acc[:, hr, :],
                                    AF.Identity, bias=bd_s[:, 0:1],
                                    scale=1.0)
                                nc.scalar.dma_start(
                                    out_d[:, s0 + h * CBLK:
                                          s0 + (h + 1) * CBLK, :], osb[:])
                        cur = nxt
                        if fillers:
                            fillers.pop(0)()
                    while fillers:
                        fillers.pop(0)()
                    flds.pop(i)
                    tris.pop(i)
                    urep.pop(i)
                    xas.pop(i)
                    if i < NBAND - 1:
                        accs[i] = acc

                def emit_evac(i):
                    """deferred into band i+1's filler stream: the ACT queue
                    is in-order, so evacuating right at band-i end would
                    park ACT on the last-matmul sem and delay band i+2's
                    om/tri chain queued behind it."""
                    s0 = i * DBLK
                    acc = accs.pop(i)
                    for h in range(2):
                        osb = pos.tile([NF, CBLK, W], FP32, tag="osb",
                                       name=f"osb{h}_{s0}")
                        nc.scalar.activation(
                            osb[:], acc[:, h * CBLK:(h + 1) * CBLK, :],
                            AF.Identity, bias=bd_s[:, 0:1], scale=1.0)
                        nc.scalar.dma_start(
                            out_d[:, s0 + h * CBLK: s0 + (h + 1) * CBLK, :],
                            osb[:])

                def emit_front(i):
                    emit_tris(i, "y")
                    emit_tris(i, "x")
                    emit_folds(i)

                def emit_u_all(i):
                    for part in range(3):
                        emit_u(i, part)

                # prelude: band 0's full chain (om/tris/folds/u) plus
                # the conv blocks it needs; om(1)+tris(1) land in band 0's
                # early fillers, om(i+2)+tris(i+2) at band i's tail
                # thereafter, so every chain link's deps are a full band
                # old by the time its queue reaches it.
                emit_xa(0)
                emit_conv1_through(5)
                emit_conv2_through(4)
                emit_om(0)
                emit_tris(0, "y")
                emit_tris(0, "x")
                emit_folds(0)
                for part in range(3):
                    emit_u(0, part)
                for i in range(NBAND):
                    if i >= 1:
                        emit_evac(i - 1)
                    if i + 1 < NBAND:
                        emit_xa(i + 1)
                    fillers = []
                    if i == 0:
                        emit_conv1_through(7)
                        fillers.append(lambda: (emit_om_pre(1),
                                                emit_om_field(1, 0),
                                                emit_tris(1, "y")))
                        fillers.append(lambda: (emit_om_field(1, 1),
                                                emit_tris(1, "x"),
                                                emit_om_field(1, 2)))
                        fillers.append(lambda: emit_folds(1))
                        fillers.append(lambda: emit_u(1, 0))
                        fillers.append(lambda: emit_u(1, 1))
                        fillers.append(lambda: emit_u(1, 2))
                        fillers.append(lambda: (emit_conv1_through(8),
                                                emit_conv2_through(6)))
                        fillers.append(lambda: (emit_om_pre(2),
                                                emit_om_field(2, 0),
                                                emit_tris(2, "y")))
                        fillers.append(lambda: (emit_om_field(2, 1),
                                                emit_tris(2, "x"),
                                                emit_om_field(2, 2)))
                    elif i + 2 < NBAND:
                        fillers.append(lambda i=i: emit_folds(i + 1))
                        fillers.append(lambda i=i: emit_u(i + 1, 0))
                        fillers.append(lambda i=i: emit_u(i + 1, 1))
                        fillers.append(lambda i=i: emit_u(i + 1, 2))
                        fillers.append(lambda i=i: emit_conv1_through(2 * i + 7))
                        fillers.append(lambda i=i: emit_conv2_through(2 * i + 5))
                        fillers.append(lambda i=i: (emit_conv1_through(2 * i + 8),
                                                    emit_conv2_through(2 * i + 6)))
                        fillers.append(lambda i=i: (emit_om_pre(i + 2),
                                                    emit_om_field(i + 2, 0),
                                                    emit_tris(i + 2, "y")))
                        fillers.append(lambda i=i: (emit_om_field(i + 2, 1),
                                                    emit_tris(i + 2, "x"),
                                                    emit_om_field(i + 2, 2)))
                    elif i + 1 < NBAND:
                        fillers.append(lambda i=i: emit_folds(i + 1))
                        fillers.append(lambda i=i: emit_u(i + 1, 0))
                        fillers.append(lambda i=i: emit_u(i + 1, 1))
                        fillers.append(lambda i=i: emit_u(i + 1, 2))
                        fillers.append(lambda i=i: emit_conv1_through(nblk1 - 1))
                        fillers.append(lambda i=i: emit_conv2_through(nblk2 - 1))
                    else:
                        fillers.append(lambda i=i: emit_conv1_through(nblk1 - 1))
                        fillers.append(lambda i=i: emit_conv2_through(nblk2 - 1))
                    emit_back(i, fillers)


def prep_weights(w_off1, b_off1, w_off2, b_off2, w_om, b_om, w_dcn, b_dcn):
    """Host-side weight layout prep (tiny tensors)."""
    f32 = np.float32

    def conv_lhst(w):  # [O, I, 3, 3] -> [I, 9, O]
        return np.ascontiguousarray(
            w.transpose(2, 3, 1, 0).reshape(TAPS, w.shape[1], w.shape[0])
            .transpose(1, 0, 2), f32)

    w1t = conv_lhst(w_off1)
    w2t = conv_lhst(w_off2)  # [64, 9, 64], tap t = (ky+1)*3 + (kx+1)
    w2p = np.empty((2 * NF, K, NF), f32)
    w2s = np.empty((NF, K, NF), f32)
    for a in range(K):  # ky = a-1
        w2p[:NF, a] = w2t[:, a * 3 + 0]      # kx=-1
        w2p[NF:, a] = w2t[:, a * 3 + 1]      # kx=0 (col+1-shifted copy)
        w2s[:, a] = w2t[:, a * 3 + 2]        # kx=+1

    # om columns ordered (f, k, g): col = f*GK + k*DG + g
    womp = np.zeros((2 * NF, K, OMC), f32)
    woms = np.zeros((NF, K, OMC), f32)
    w_om_r = w_om.reshape(3, DG, TAPS, NF, K, K)  # [f, g, k, i, ky, kx]
    for f in range(3):
        for g in range(DG):
            for k in range(TAPS):
                col = f * GK + k * DG + g
                for a in range(K):
                    womp[:NF, a, col] = w_om_r[f, g, k, :, a, 0]
                    womp[NF:, a, col] = w_om_r[f, g, k, :, a, 1]
                    woms[:, a, col] = w_om_r[f, g, k, :, a, 2]

    # deform weights: wdt[k*8+g, c, o] = w_dcn[o, g*8+c, ky, kx]
    wd_r = w_dcn.reshape(NF, DG, CG, K, K)  # [o, g, c, ky, kx]
    wdt = np.empty((GK, CG, NF), f32)
    for k in range(TAPS):
        ky, kx = _tap(k)
        for g in range(DG):
            wdt[k * DG + g] = wd_r[:, g, :, ky + 1, kx + 1].T  # [c, o]
    # A: partition p=(k*16+g*2+ch), lhsT per cb: wda[p, cb, o]
    wda = np.empty((128, 4, NF), f32)
    for k in range(8):
        for g in range(DG):
            for ch in range(2):
                wda[k * 16 + g * 2 + ch] = wdt[k * DG + g][ch * 4: ch * 4 + 4, :]
    # B: tap 8, partition q=(g*8+c)
    wdb = np.empty((NF, NF), f32)
    for g in range(DG):
        for c in range(CG):
            wdb[g * CG + c] = wdt[8 * DG + g, c, :]

    bom = np.empty((OMC, 1), f32)
    bor = b_om.reshape(3, DG, TAPS)
    for f in range(3):
        for k in range(TAPS):
            for g in range(DG):
                bom[f * GK + k * DG + g, 0] = bor[f, g, k]

    import ml_dtypes
    bf = ml_dtypes.bfloat16
    return dict(
        w1t=w1t, w2p=w2p, w2s=w2s,
        womp=np.ascontiguousarray(womp), woms=np.ascontiguousarray(woms),
        wda=np.ascontiguousarray(wda.astype(bf)),
        wdb=np.ascontiguousarray(wdb.astype(bf)), bom=bom,
        b1=np.ascontiguousarray(b_off1[:, None], f32),
        b2=np.ascontiguousarray(b_off2[:, None], f32),
        bd=np.ascontiguousarray(b_dcn[:, None], f32),
    )


def prep_core_inputs(nbr, ref, weights_map):
    """Per-core input dicts: 8 cores = (sample b, row-half)."""
    import ml_dtypes
    bf = ml_dtypes.bfloat16
    in_maps = []
    for core in range(NCORES):
        b, half = core // 2, core % 2
        r0 = half * NR
        xin_full = np.concatenate([nbr[b], ref[b]], axis=0)
        xpad = np.pad(xin_full, ((0, 0), (3, 3), (1, 1)))
        xin = np.ascontiguousarray(
            xpad[:, r0: r0 + NR + 6, :].astype(bf))
        # tap-shifted bf16 replicas, per band (halo rows duplicated):
        #   base[k][c_img, jj, w] = image[c_img, r0+s0-1+jj+ky, w-2+kx]
        pad3 = np.pad(nbr[b], ((0, 0), (3, 3), (3, 3))).astype(bf)
        nbxa = np.empty((NBAND, 128, 4, DBLK + 2, W + 4), bf)
        nbxb = np.empty((NBAND, NF, DBLK + 2, W + 4), bf)
        for i in range(NBAND):
            s0 = i * DBLK
            for k in range(TAPS):
                ky, kx = _tap(k)
                src = pad3[:, r0 + s0 + 2 + ky: r0 + s0 + 2 + ky + DBLK + 2,
                           1 + kx: 1 + kx + W + 4]
                if k < 8:
                    for g in range(DG):
                        for ch in range(2):
                            nbxa[i, k * 16 + g * 2 + ch] = \
                                src[g * CG + ch * 4: g * CG + ch * 4 + 4]
                else:
                    nbxb[i] = src
        m = dict(weights_map)
        m["xin"] = xin
        m["nbxa"] = np.ascontiguousarray(nbxa)
        m["nbxb"] = np.ascontiguousarray(nbxb)
        y1 = np.arange(r0 - 3, r0 + NR + 3)
        m["rmask1"] = np.broadcast_to(
            ((y1 >= 0) & (y1 < H)).astype(bf)[None, :, None],
            (2 * NF, NR + 6, 1)).copy()
        y2 = np.arange(r0 - 2, r0 + NR + 2)
        m["rmask2"] = np.broadcast_to(
            ((y2 >= 0) & (y2 < H)).astype(bf)[None, :, None],
            (2 * NF, NR + 4, 1)).copy()
        in_maps.append(m)
    return in_maps


_CACHE = {}


def kernel(nbr, ref, w_off1, b_off1, w_off2, b_off2, w_om, b_om, w_dcn, b_dcn):
    nbr = np.asarray(nbr, np.float32)
    ref = np.asarray(ref, np.float32)
    if "nc" not in _CACHE:
        _CACHE["nc"] = build_program()
    nc = _CACHE["nc"]
    wmap = prep_weights(np.asarray(w_off1), np.asarray(b_off1),
                        np.asarray(w_off2), np.asarray(b_off2),
                        np.asarray(w_om), np.asarray(b_om),
                        np.asarray(w_dcn), np.asarray(b_dcn))
    in_maps = prep_core_inputs(nbr, ref, wmap)
    res = bass_utils.run_bass_kernel_spmd(nc, in_maps, list(range(NCORES)))
    out = np.empty((B, NF, H, W), np.float32)
    for core in range(NCORES):
        b, half = core // 2, core % 2
        out[b, :, half * NR:(half + 1) * NR, :] = res.results[core]["out"]
    return out
